# revision 24
# baseline (speedup 1.0000x reference)
"""Trainium2 Bass kernel for nn_BatchedTrilLinear.

y[n, b*64:(b+1)*64] = x[n, b*64:(b+1)*64] @ L_b.T  for b in range(512),
with L_b = tril(W_b, -1) + diag(exp(diag(W_b))).

Sharding (default): block-parallel — core k owns blocks [64k, 64(k+1))
(a 4096-column slice of x/y, all 4096 rows) plus its 1 MB slice of the
host-precomputed weights; no cross-device communication. A data-parallel
N-sharded build (build()/_body) is kept for reference.

Host prep (inside kernel(), before device dispatch):
  - x cast to bf16 (input read traffic halves; bf16 transposes run at
    1 cycle/row on PE vs 2 for fp32)
  - weights -> L -> per-strip block-diagonal transposed tiles in bf16:
    bd[64j+i, s, 64j'+o] = L_{2s+j}[o, i] if j==j' else 0   (strip s = 2 blocks)
  - y comes back bf16 and is upcast to f32 on host.
  End-to-end bf16 rel error 8.2e-3 vs the 2e-2 gate.
  Per-core HBM traffic: 32 MB x + 32 MB y + 1 MB bd = 65 MB.

Per-core device dataflow (2-pass PE, no transpose-back), strips processed
in PAIRS so each PSUM drain is one instruction (halves DVE/ACT fixed costs):
  A(pair): 8 PE transposes x chunks [128n,128c] -> psx2 [128, 2, NSB] (bf16);
           one DVE copy psx2 -> xt2 in SBUF (bf16 PSUM reads are 2x on DVE)
  B(pair): 8 PE matmuls with STATIONARY = xt2 chunk [128 (j,i), 128 n] and
           MOVING = bd_s [128 (j,i), 128 (j,o)] -> psy2 [128 n, 2, NT, 128 o]
           lands in NATURAL row layout (out = xt.T @ bd = x @ L^T blocks);
           one ACT copy-cast psy2 (f32) -> yg (bf16), every 7th pair on DVE
  x/y move in SG-strip groups (4 MB DMAs, 4 KB lines); bd fully resident.
  Sim profile: DMA 96% busy (roofline), PE/DVE/ACT balanced at ~56%.
"""
import os
import sys
from contextlib import ExitStack

for _p in ("/opt/trn_rl_repo",):
    if os.path.isdir(_p) and _p not in sys.path:
        sys.path.insert(0, _p)

import numpy as np

N_FULL = 4096
B_FULL = 512
D = 64
NCORES = 8
NS = N_FULL // NCORES        # rows per core

_built = {}


def _body(ctx, tc, y_d, x_d, bd_d, *, NS, B, SG, SC, repeat=1,
          split_copies=False, compact_bd=False, xg_bufs=2, yg_bufs=2):
    import concourse.mybir as mybir
    from concourse.masks import make_identity

    nc = tc.nc
    f32 = mybir.dt.float32
    bf16 = mybir.dt.bfloat16
    S = B // 2               # strips (2 blocks each)
    NT = NS // 128           # n-tiles
    G = S // SG              # strip groups (x/y DMA granularity)
    CG = SG * 128            # columns per group
    BC = S // SC             # bd chunks

    const_pool = ctx.enter_context(tc.tile_pool(name="const", bufs=1))
    xg_pool = ctx.enter_context(tc.tile_pool(name="xg", bufs=xg_bufs))
    yg_pool = ctx.enter_context(tc.tile_pool(name="yg", bufs=yg_bufs))
    bd_pool = ctx.enter_context(tc.tile_pool(name="bd", bufs=2))
    xt_pool = ctx.enter_context(tc.tile_pool(name="xt", bufs=3))
    psx_pool = ctx.enter_context(tc.tile_pool(name="psx", bufs=3, space="PSUM"))
    psy_pool = ctx.enter_context(tc.tile_pool(name="psy", bufs=3, space="PSUM"))
    if compact_bd:
        bdx_pool = ctx.enter_context(tc.tile_pool(name="bdx", bufs=3))

    ident_f = const_pool.tile([128, 128], f32)
    make_identity(nc, ident_f)
    ident = const_pool.tile([128, 128], bf16)
    nc.gpsimd.tensor_copy(ident[:], ident_f[:])

    if compact_bd:
        # bdm[64j+i, u, o] = 1 if u == j else 0 (block-diag expander mask)
        bdm = const_pool.tile([128, 2, D], bf16)
        nc.gpsimd.memset(bdm[:], 0.0)
        for j in range(2):
            nc.gpsimd.memset(bdm[64 * j:64 * j + 64, j, :], 1.0)

    x_view = x_d.rearrange("(t p) c -> p t c", p=128)     # [128, NT, C]
    y_view = y_d.rearrange("(t p) c -> p t c", p=128)

    xg_tiles = {}
    bd_tiles = {}
    yg_tiles = {}

    def fetch_group(g):
        xg = xg_pool.tile([128, NT, CG], bf16, tag="xg")
        nc.sync.dma_start(xg[:], x_view[:, :, g * CG:(g + 1) * CG])
        xg_tiles[g] = xg

    def fetch_bd(c):
        if compact_bd:
            bd = bd_pool.tile([128, SC, D], bf16, tag="bd")
        else:
            bd = bd_pool.tile([128, SC, 128], bf16, tag="bd")
        nc.sync.dma_start(bd[:], bd_d[:, c * SC:(c + 1) * SC, :])
        bd_tiles[c] = bd

    def stage_a(s):
        g, sl = divmod(s, SG)
        if sl == 0 and g + 1 < G:
            fetch_group(g + 1)
        if s % SC == SC // 2 and s // SC + 1 < BC:
            fetch_bd(s // SC + 1)
        xg = xg_tiles[g]
        psx = psx_pool.tile([128, NS], bf16, tag="psx")
        for t in range(NT):
            nc.tensor.matmul(psx[:, t * 128:(t + 1) * 128],
                             lhsT=xg[:, t, sl * 128:(sl + 1) * 128],
                             rhs=ident[:], is_transpose=True,
                             start=(t == 0), stop=(t == NT - 1))
        xt = xt_pool.tile([128, NS], bf16, tag="xt")
        h = NS // 2
        if split_copies:
            if s % 2 == 0:
                nc.vector.tensor_copy(xt[:, :h], psx[:, :h])
                nc.scalar.copy(xt[:, h:], psx[:, h:])
            else:
                nc.scalar.copy(xt[:, :h], psx[:, :h])
                nc.vector.tensor_copy(xt[:, h:], psx[:, h:])
        elif s % 2 == 0:
            nc.vector.tensor_copy(xt[:], psx[:])
        else:
            nc.scalar.copy(xt[:], psx[:])
        bdx = None
        if compact_bd:
            lt = bd_tiles[s // SC]
            slc = s % SC
            bdx = bdx_pool.tile([128, 2, D], bf16, tag="bdx")
            nc.gpsimd.tensor_tensor(
                bdx[:], lt[:, slc, None, :].to_broadcast((128, 2, D)),
                bdm[:], op=mybir.AluOpType.mult)
        return xt, bdx

    def stage_b(s, xt, bdx):
        g, sl = divmod(s, SG)
        if sl == 0:
            yg = yg_pool.tile([128, NT, CG], bf16, tag="yg")
            yg_tiles[g] = yg
        yg = yg_tiles[g]
        if compact_bd:
            rhs = bdx.rearrange("p u c -> p (u c)")
        else:
            rhs = bd_tiles[s // SC][:, s % SC, :]
        psy = psy_pool.tile([128, NT, 128], f32, tag="psy")
        for t in range(NT):
            nc.tensor.matmul(psy[:, t, :],
                             lhsT=xt[:, t * 128:(t + 1) * 128],
                             rhs=rhs)
        dst = yg[:, :, sl * 128:(sl + 1) * 128]
        hh = NT // 2
        if split_copies:
            if s % 2 == 0:
                nc.scalar.copy(dst[:, :hh, :], psy[:, :hh, :])
                nc.vector.tensor_copy(dst[:, hh:, :], psy[:, hh:, :])
            else:
                nc.vector.tensor_copy(dst[:, :hh, :], psy[:, :hh, :])
                nc.scalar.copy(dst[:, hh:, :], psy[:, hh:, :])
        elif s % 2 == 0:
            nc.scalar.copy(dst, psy[:])
        else:
            nc.vector.tensor_copy(dst, psy[:])
        if sl == SG - 1:
            nc.sync.dma_start(y_view[:, :, g * CG:(g + 1) * CG], yg[:])

    for _rep in range(repeat):
        xg_tiles.clear()
        bd_tiles.clear()
        yg_tiles.clear()
        fetch_bd(0)
        fetch_group(0)
        prev = None
        for s in range(S):
            cur = stage_a(s)
            if prev is not None:
                stage_b(s - 1, *prev)
            prev = cur
        stage_b(S - 1, *prev)


def _body_cs(ctx, tc, y_d, x_d, bd_d, *, NT_ALL, B, SG, repeat=1,
             xg_bufs=3, yg_bufs=3, NSB=512):
    """Column-sharded body: this core owns B blocks (all N rows), bd resident.

    x_d/y_d are [NT_ALL*128, B*D]; rows processed in bands of NSB."""
    import concourse.mybir as mybir
    from concourse.masks import make_identity

    nc = tc.nc
    f32 = mybir.dt.float32
    bf16 = mybir.dt.bfloat16
    S = B // 2               # strips (2 blocks each)
    NT = NSB // 128          # n-tiles per band
    NB = NT_ALL // NT        # bands
    G = S // SG              # strip groups per band
    CG = SG * 128            # columns per group

    const_pool = ctx.enter_context(tc.tile_pool(name="const", bufs=1))
    xg_pool = ctx.enter_context(tc.tile_pool(name="xg", bufs=xg_bufs))
    yg_pool = ctx.enter_context(tc.tile_pool(name="yg", bufs=yg_bufs))
    bd_pool = ctx.enter_context(tc.tile_pool(name="bd", bufs=1))
    xt_pool = ctx.enter_context(tc.tile_pool(name="xt", bufs=3))
    psx_pool = ctx.enter_context(tc.tile_pool(name="psx", bufs=3, space="PSUM"))
    psy_pool = ctx.enter_context(tc.tile_pool(name="psy", bufs=3, space="PSUM"))

    ident_f = const_pool.tile([128, 128], f32)
    make_identity(nc, ident_f)
    ident = const_pool.tile([128, 128], bf16)
    nc.gpsimd.tensor_copy(ident[:], ident_f[:])

    x_view = x_d.rearrange("(b t p) c -> p b t c", p=128, t=NT)
    y_view = y_d.rearrange("(b t p) c -> p b t c", p=128, t=NT)

    xg_tiles = {}
    yg_tiles = {}
    bd_tile = [None]

    def fetch_group(b, g):
        xg = xg_pool.tile([128, NT, CG], bf16, tag="xg")
        nc.sync.dma_start(xg[:], x_view[:, b, :, g * CG:(g + 1) * CG])
        xg_tiles[(b, g)] = xg

    def stage_a(b, s):
        g, sl = divmod(s, SG)
        if sl == 0:
            if g + 1 < G:
                fetch_group(b, g + 1)
            elif b + 1 < NB:
                fetch_group(b + 1, 0)
        xg = xg_tiles[(b, g)]
        psx = psx_pool.tile([128, NSB], bf16, tag="psx")
        for t in range(NT):
            nc.tensor.matmul(psx[:, t * 128:(t + 1) * 128],
                             lhsT=xg[:, t, sl * 128:(sl + 1) * 128],
                             rhs=ident[:], is_transpose=True,
                             start=(t == 0), stop=(t == NT - 1))
        xt = xt_pool.tile([128, NSB], bf16, tag="xt")
        if s % 2 == 0:
            nc.vector.tensor_copy(xt[:], psx[:])
        else:
            nc.scalar.copy(xt[:], psx[:])
        return xt

    def stage_b(b, s, xt):
        g, sl = divmod(s, SG)
        if sl == 0:
            yg = yg_pool.tile([128, NT, CG], bf16, tag="yg")
            yg_tiles[(b, g)] = yg
        yg = yg_tiles[(b, g)]
        psy = psy_pool.tile([128, NT, 128], f32, tag="psy")
        for t in range(NT):
            nc.tensor.matmul(psy[:, t, :],
                             lhsT=xt[:, t * 128:(t + 1) * 128],
                             rhs=bd_tile[0][:, s, :])
        dst = yg[:, :, sl * 128:(sl + 1) * 128]
        if s % 2 == 0:
            nc.scalar.copy(dst, psy[:])
        else:
            nc.vector.tensor_copy(dst, psy[:])
        if sl == SG - 1:
            nc.sync.dma_start(y_view[:, b, :, g * CG:(g + 1) * CG], yg[:])

    for _rep in range(repeat):
        xg_tiles.clear()
        yg_tiles.clear()
        fetch_group(0, 0)
        bd = bd_pool.tile([128, S, 128], bf16, tag="bd")
        nc.sync.dma_start(bd[:], bd_d[:])
        bd_tile[0] = bd
        prev = None
        for b in range(NB):
            for s in range(S):
                cur = (b, s, stage_a(b, s))
                if prev is not None:
                    stage_b(prev[0], prev[1], prev[2])
                prev = cur
        stage_b(prev[0], prev[1], prev[2])


def build(NS=NS, B=B_FULL, SG=8, SC=32, repeat=1, split_copies=False,
          compact_bd=True, xg_bufs=3, yg_bufs=3):
    key = (NS, B, SG, SC, repeat, split_copies, compact_bd, xg_bufs, yg_bufs)
    if key in _built:
        return _built[key]
    import concourse.tile as tile
    import concourse.mybir as mybir
    from concourse import bacc

    bf16 = mybir.dt.bfloat16
    C = B * D
    S = B // 2
    nc = bacc.Bacc("TRN2", target_bir_lowering=False, debug=False)
    x_d = nc.dram_tensor("x", [NS, C], bf16, kind="ExternalInput").ap()
    bd_shape = [128, S, D] if compact_bd else [128, S, 128]
    bd_d = nc.dram_tensor("bd", bd_shape, bf16, kind="ExternalInput").ap()
    y_d = nc.dram_tensor("y", [NS, C], bf16, kind="ExternalOutput").ap()
    with tile.TileContext(nc) as tc, ExitStack() as ctx:
        _body(ctx, tc, y_d, x_d, bd_d, NS=NS, B=B, SG=SG, SC=SC, repeat=repeat,
              split_copies=split_copies, compact_bd=compact_bd,
              xg_bufs=xg_bufs, yg_bufs=yg_bufs)
    nc.compile()
    _built[key] = nc
    return nc


def _body_cs2(ctx, tc, y_d, x_d, bd_d, *, NT_ALL, B, SG, repeat=1,
              xg_bufs=3, yg_bufs=3, NSB=512, dve_frac=7, ragged=1):
    """Paired-strip column-sharded body: strips processed two at a time so
    each PSUM->SBUF drain is one instruction (halves DVE/ACT fixed costs).

    xt2 copies always on DVE (bf16 PSUM reads are 2x there); psy2 copies on
    ACT except every dve_frac-th pair, which goes to DVE for balance."""
    import concourse.mybir as mybir
    from concourse.masks import make_identity

    nc = tc.nc
    f32 = mybir.dt.float32
    bf16 = mybir.dt.bfloat16
    S = B // 2               # strips (2 blocks each)
    NT = NSB // 128          # n-tiles per band
    NB = NT_ALL // NT        # bands
    G = S // SG              # strip groups per band
    CG = SG * 128            # columns per group
    assert S % 2 == 0 and SG % 2 == 0

    const_pool = ctx.enter_context(tc.tile_pool(name="const", bufs=1))
    xg_pool = ctx.enter_context(tc.tile_pool(name="xg", bufs=xg_bufs))
    yg_pool = ctx.enter_context(tc.tile_pool(name="yg", bufs=yg_bufs))
    bd_pool = ctx.enter_context(tc.tile_pool(name="bd", bufs=1))
    xt_pool = ctx.enter_context(tc.tile_pool(name="xt", bufs=3))
    psx_pool = ctx.enter_context(tc.tile_pool(name="psx", bufs=3, space="PSUM"))
    psy_pool = ctx.enter_context(tc.tile_pool(name="psy", bufs=2, space="PSUM"))

    ident_f = const_pool.tile([128, 128], f32)
    make_identity(nc, ident_f)
    ident = const_pool.tile([128, 128], bf16)
    nc.gpsimd.tensor_copy(ident[:], ident_f[:])

    x_view = x_d.rearrange("(b t p) c -> p b t c", p=128, t=NT)
    y_view = y_d.rearrange("(b t p) c -> p b t c", p=128, t=NT)

    # per-band group plans [(start_strip, width_strips)]; ragged halves the
    # first groups of band 0 and the last groups of the final band so the
    # pipeline-fill DMA (and final drain) is half-sized.
    def make_plan(b):
        plan = [(i * SG, SG) for i in range(G)]
        if ragged and SG >= 4:
            h = SG // 2
            if ragged >= 2 and SG >= 8:
                q = SG // 4
                if b == 0:
                    plan = [(0, q), (q, q), (2 * q, h)] + plan[1:]
                if b == NB - 1:
                    plan = plan[:-1] + [(S - SG, h), (S - h, q), (S - q, q)]
            else:
                if b == 0:
                    plan = [(0, h), (h, h)] + plan[1:]
                if b == NB - 1:
                    plan = plan[:-1] + [(S - SG, h), (S - h, h)]
        return plan

    plans = [make_plan(b) for b in range(NB)]
    smap = [{s: gi for gi, (c0, w) in enumerate(p)
             for s in range(c0, c0 + w)} for p in plans]

    xg_tiles = {}
    yg_tiles = {}
    bd_tile = [None]

    def fetch_group(b, gi):
        c0, w = plans[b][gi]
        xg = xg_pool.tile([128, NT, w * 128], bf16, tag="xg")
        nc.sync.dma_start(xg[:], x_view[:, b, :, c0 * 128:(c0 + w) * 128])
        xg_tiles[(b, gi)] = xg

    def stage_a(b, sp):
        """Transpose strips 2sp, 2sp+1 into one psx2; one copy to xt2."""
        psx = psx_pool.tile([128, 2, NSB], bf16, tag="psx")
        for j in range(2):
            s = 2 * sp + j
            gi = smap[b][s]
            c0, w = plans[b][gi]
            sl = s - c0
            if sl == 0:
                if gi + 1 < len(plans[b]):
                    fetch_group(b, gi + 1)
                elif b + 1 < NB:
                    fetch_group(b + 1, 0)
            xg = xg_tiles[(b, gi)]
            for t in range(NT):
                nc.tensor.matmul(psx[:, j, t * 128:(t + 1) * 128],
                                 lhsT=xg[:, t, sl * 128:(sl + 1) * 128],
                                 rhs=ident[:], is_transpose=True,
                                 start=(j == 0 and t == 0),
                                 stop=(j == 1 and t == NT - 1))
        xt = xt_pool.tile([128, 2, NSB], bf16, tag="xt")
        nc.vector.tensor_copy(xt[:], psx[:])
        return xt

    def stage_b(b, sp, xt):
        psy = psy_pool.tile([128, 2, NT, 128], f32, tag="psy")
        for j in range(2):
            s = 2 * sp + j
            gi = smap[b][s]
            c0, w = plans[b][gi]
            if s == c0:
                yg = yg_pool.tile([128, NT, w * 128], bf16, tag="yg")
                yg_tiles[(b, gi)] = yg
            for t in range(NT):
                nc.tensor.matmul(psy[:, j, t, :],
                                 lhsT=xt[:, j, t * 128:(t + 1) * 128],
                                 rhs=bd_tile[0][:, s, :])
        g0 = smap[b][2 * sp]
        c0, w = plans[b][g0]
        sl0 = 2 * sp - c0
        yg = yg_tiles[(b, g0)]
        dst = yg[:, :, sl0 * 128:(sl0 + 2) * 128].rearrange(
            "p t (j c) -> p j t c", j=2)
        if sp % dve_frac == 0:
            nc.vector.tensor_copy(dst, psy[:])
        else:
            nc.scalar.copy(dst, psy[:])
        if sl0 + 2 == w:
            nc.sync.dma_start(y_view[:, b, :, c0 * 128:(c0 + w) * 128], yg[:])

    for _rep in range(repeat):
        xg_tiles.clear()
        yg_tiles.clear()
        fetch_group(0, 0)
        bd = bd_pool.tile([128, S, 128], bf16, tag="bd")
        nc.sync.dma_start(bd[:], bd_d[:])
        bd_tile[0] = bd
        prev = None
        for b in range(NB):
            for sp in range(S // 2):
                cur = (b, sp, stage_a(b, sp))
                if prev is not None:
                    stage_b(prev[0], prev[1], prev[2])
                prev = cur
        stage_b(prev[0], prev[1], prev[2])


def _body_diag(ctx, tc, y_d, x_d, bd_d, *, NT_ALL, B, SG, repeat=1,
               xg_bufs=3, yg_bufs=3, NSB=512, mode="dma", dve_frac=7):
    """Diagnostic bodies: mode='dma' issues only the DMA traffic of the real
    kernel; mode='nodma' runs the full compute/copy pipeline with all
    dma_start calls elided (tiles allocated, never filled/drained)."""
    import concourse.mybir as mybir
    from concourse.masks import make_identity

    nc = tc.nc
    f32 = mybir.dt.float32
    bf16 = mybir.dt.bfloat16
    S = B // 2
    NT = NSB // 128
    NB = NT_ALL // NT
    G = S // SG
    CG = SG * 128

    const_pool = ctx.enter_context(tc.tile_pool(name="const", bufs=1))
    xg_pool = ctx.enter_context(tc.tile_pool(name="xg", bufs=xg_bufs))
    yg_pool = ctx.enter_context(tc.tile_pool(name="yg", bufs=yg_bufs))
    bd_pool = ctx.enter_context(tc.tile_pool(name="bd", bufs=1))
    xt_pool = ctx.enter_context(tc.tile_pool(name="xt", bufs=3))
    psx_pool = ctx.enter_context(tc.tile_pool(name="psx", bufs=3, space="PSUM"))
    psy_pool = ctx.enter_context(tc.tile_pool(name="psy", bufs=2, space="PSUM"))

    ident_f = const_pool.tile([128, 128], f32)
    make_identity(nc, ident_f)
    ident = const_pool.tile([128, 128], bf16)
    nc.gpsimd.tensor_copy(ident[:], ident_f[:])

    x_view = x_d.rearrange("(b t p) c -> p b t c", p=128, t=NT)
    y_view = y_d.rearrange("(b t p) c -> p b t c", p=128, t=NT)

    do_dma = mode == "dma"
    xg_tiles = {}
    yg_tiles = {}
    bd_tile = [None]
    if mode == "dma":
        yg0 = const_pool.tile([128, NT, CG], bf16)
        nc.gpsimd.memset(yg0[:], 0.0)
    xg0 = bd0 = None
    if mode == "nodma":
        xg0 = const_pool.tile([128, NT, CG], bf16)
        nc.gpsimd.memset(xg0[:], 0.0)
        bd0 = const_pool.tile([128, S, 128], bf16)
        nc.gpsimd.memset(bd0[:], 0.0)

    def fetch_group(b, g):
        if not do_dma:
            xg_tiles[(b, g)] = xg0
            return
        xg = xg_pool.tile([128, NT, CG], bf16, tag="xg")
        nc.sync.dma_start(xg[:], x_view[:, b, :, g * CG:(g + 1) * CG])
        xg_tiles[(b, g)] = xg

    def stage_a(b, sp):
        psx = psx_pool.tile([128, 2, NSB], bf16, tag="psx")
        for j in range(2):
            s = 2 * sp + j
            g, sl = divmod(s, SG)
            if sl == 0:
                if g + 1 < G:
                    fetch_group(b, g + 1)
                elif b + 1 < NB:
                    fetch_group(b + 1, 0)
            xg = xg_tiles[(b, g)]
            for t in range(NT):
                nc.tensor.matmul(psx[:, j, t * 128:(t + 1) * 128],
                                 lhsT=xg[:, t, sl * 128:(sl + 1) * 128],
                                 rhs=ident[:], is_transpose=True,
                                 start=(j == 0 and t == 0),
                                 stop=(j == 1 and t == NT - 1))
        xt = xt_pool.tile([128, 2, NSB], bf16, tag="xt")
        nc.vector.tensor_copy(xt[:], psx[:])
        return xt

    def stage_b(b, sp, xt):
        psy = psy_pool.tile([128, 2, NT, 128], f32, tag="psy")
        for j in range(2):
            s = 2 * sp + j
            g, sl = divmod(s, SG)
            if sl == 0:
                yg = yg_pool.tile([128, NT, CG], bf16, tag="yg")
                yg_tiles[(b, g)] = yg
            for t in range(NT):
                nc.tensor.matmul(psy[:, j, t, :],
                                 lhsT=xt[:, j, t * 128:(t + 1) * 128],
                                 rhs=bd_tile[0][:, s, :])
        g0, sl0 = divmod(2 * sp, SG)
        yg = yg_tiles[(b, g0)]
        dst = yg[:, :, sl0 * 128:(sl0 + 2) * 128].rearrange(
            "p t (j c) -> p j t c", j=2)
        if sp % dve_frac == 0:
            nc.vector.tensor_copy(dst, psy[:])
        else:
            nc.scalar.copy(dst, psy[:])
        if do_dma and sl0 + 2 == SG:
            nc.sync.dma_start(y_view[:, b, :, g0 * CG:(g0 + 1) * CG], yg[:])

    for _rep in range(repeat):
        xg_tiles.clear()
        yg_tiles.clear()
        fetch_group(0, 0)
        if do_dma:
            bd = bd_pool.tile([128, S, 128], bf16, tag="bd")
            nc.sync.dma_start(bd[:], bd_d[:])
            bd_tile[0] = bd
        else:
            bd_tile[0] = bd0
        if mode == "dma":
            for b in range(NB):
                for g in range(G):
                    if (b, g) != (0, 0):
                        fetch_group(b, g)
                    nc.sync.dma_start(y_view[:, b, :, g * CG:(g + 1) * CG],
                                      yg0[:])
            continue
        prev = None
        for b in range(NB):
            for sp in range(S // 2):
                cur = (b, sp, stage_a(b, sp))
                if prev is not None:
                    stage_b(prev[0], prev[1], prev[2])
                prev = cur
        stage_b(prev[0], prev[1], prev[2])


def build_cs(N=N_FULL, BL=B_FULL // NCORES, SG=16, repeat=1, xg_bufs=4,
             yg_bufs=4, NSB=512, paired=1, dve_frac=7, diag=None,
             ragged=1):
    key = ("cs", N, BL, SG, repeat, xg_bufs, yg_bufs, NSB, paired, dve_frac,
           diag, ragged)
    if key in _built:
        return _built[key]
    import concourse.tile as tile
    import concourse.mybir as mybir
    from concourse import bacc

    bf16 = mybir.dt.bfloat16
    C = BL * D
    S = BL // 2
    nc = bacc.Bacc("TRN2", target_bir_lowering=False, debug=False)
    x_d = nc.dram_tensor("x", [N, C], bf16, kind="ExternalInput").ap()
    bd_d = nc.dram_tensor("bd", [128, S, 128], bf16, kind="ExternalInput").ap()
    y_d = nc.dram_tensor("y", [N, C], bf16, kind="ExternalOutput").ap()
    body = _body_cs2 if paired else _body_cs
    kw = dict(dve_frac=dve_frac) if paired else {}
    if paired and not diag:
        kw["ragged"] = ragged
    if diag:
        body = _body_diag
        kw = dict(mode=diag, dve_frac=dve_frac)
    with tile.TileContext(nc) as tc, ExitStack() as ctx:
        body(ctx, tc, y_d, x_d, bd_d, NT_ALL=N // 128, B=BL, SG=SG,
             repeat=repeat, xg_bufs=xg_bufs, yg_bufs=yg_bufs, NSB=NSB, **kw)
    nc.compile()
    _built[key] = nc
    return nc


def make_core_inputs(xb, bd, col_shard=False):
    """Per-core input dicts from full host-prepped arrays."""
    if not col_shard:
        ns = xb.shape[0] // NCORES
        return [{"x": xb[k * ns:(k + 1) * ns], "bd": bd} for k in range(NCORES)]
    cl = xb.shape[1] // NCORES
    sl = bd.shape[1] // NCORES
    return [{"x": np.ascontiguousarray(xb[:, k * cl:(k + 1) * cl]),
             "bd": np.ascontiguousarray(bd[:, k * sl:(k + 1) * sl, :])}
            for k in range(NCORES)]


def unshard_y(ys, col_shard=False):
    return np.concatenate(ys, axis=1 if col_shard else 0)


def host_prep(x, weights, B=B_FULL, compact_bd=True):
    """x [N, B*D] f32, weights [B, D, D] f32 ->
    (x bf16 [N, B*D], bd bf16 [128, B//2, 128] or compact [128, B//2, 64])."""
    import ml_dtypes

    bf = ml_dtypes.bfloat16
    S = B // 2
    w = np.ascontiguousarray(weights, dtype=np.float32)
    L = np.tril(w, -1)
    idx = np.arange(D)
    L[:, idx, idx] = np.exp(w[:, idx, idx])
    # LTr[j, i, s, o] = L[2s+j, o, i]
    LTr = L.reshape(S, 2, D, D).transpose(1, 3, 0, 2)
    if compact_bd:
        # lt[64j+i, s, o] = L_{2s+j}[o, i]
        bd = np.ascontiguousarray(
            LTr.reshape(2 * D, S, D).astype(bf))
    else:
        bd = np.zeros((2, D, S, 2, D), dtype=bf)
        bd[0, :, :, 0, :] = LTr[0].astype(bf)
        bd[1, :, :, 1, :] = LTr[1].astype(bf)
        bd = np.ascontiguousarray(bd.reshape(2 * D, S, 2 * D))
    xb = np.ascontiguousarray(np.asarray(x, dtype=np.float32)).astype(bf)
    return xb, bd


def _pin_compile_cache(extra=""):
    import hashlib
    with open(os.path.abspath(__file__), "rb") as f:
        h = hashlib.sha256(f.read() + extra.encode()).hexdigest()[:16]
    os.environ["NEURON_COMPILE_CACHE_URL"] = f"/tmp/neuron_cache_{h}"


def run(x, weights, trace=False, col_shard=True, **opts):
    from concourse import bass_utils

    _pin_compile_cache()

    x = np.asarray(x)
    weights = np.asarray(weights)
    assert x.shape == (N_FULL, B_FULL * D), x.shape
    assert weights.shape == (B_FULL, D, D), weights.shape
    xb, bd = host_prep(x, weights,
                       compact_bd=(not col_shard) and opts.get("compact_bd", True))

    nc = build_cs(**opts) if col_shard else build(**opts)
    in_maps = make_core_inputs(xb, bd, col_shard=col_shard)
    res = bass_utils.run_bass_kernel_spmd(
        nc, in_maps, core_ids=list(range(NCORES)), trace=trace)
    y = unshard_y([np.asarray(res.results[k]["y"]) for k in range(NCORES)],
                  col_shard=col_shard)
    return y.astype(np.float32), res


def kernel(x, weights):
    y, _ = run(x, weights)
    return y


# revision 27
# speedup vs baseline: 1.0563x; 1.0563x over previous
"""Trainium2 Bass kernel for nn_BatchedTrilLinear.

y[n, b*64:(b+1)*64] = x[n, b*64:(b+1)*64] @ L_b.T  for b in range(512),
with L_b = tril(W_b, -1) + diag(exp(diag(W_b))).

Sharding (default): block-parallel — core k owns blocks [64k, 64(k+1))
(a 4096-column slice of x/y, all 4096 rows) plus its 1 MB slice of the
host-precomputed weights; no cross-device communication. A data-parallel
N-sharded build (build()/_body) is kept for reference.

Host prep (inside kernel(), before device dispatch):
  - x cast to bf16 (input read traffic halves; bf16 transposes run at
    1 cycle/row on PE vs 2 for fp32)
  - weights -> L -> per-strip block-diagonal transposed tiles in bf16:
    bd[64j+i, s, 64j'+o] = L_{2s+j}[o, i] if j==j' else 0   (strip s = 2 blocks)
  - y comes back bf16 and is upcast to f32 on host.
  End-to-end bf16 rel error 8.2e-3 vs the 2e-2 gate.
  Per-core HBM traffic: 32 MB x + 32 MB y + 1 MB bd = 65 MB.

Per-core device dataflow (2-pass PE, no transpose-back), strips processed
in PAIRS so each PSUM drain is one instruction (halves DVE/ACT fixed costs):
  A(pair): 8 PE transposes x chunks [128n,128c] -> psx2 [128, 2, NSB] (bf16);
           one DVE copy psx2 -> xt2 in SBUF (bf16 PSUM reads are 2x on DVE)
  B(pair): 8 PE matmuls with STATIONARY = xt2 chunk [128 (j,i), 128 n] and
           MOVING = bd_s [128 (j,i), 128 (j,o)] -> psy2 [128 n, 2, NT, 128 o]
           lands in NATURAL row layout (out = xt.T @ bd = x @ L^T blocks);
           one ACT copy-cast psy2 (f32) -> yg (bf16), every 7th pair on DVE
  x/y move in SG-strip groups (4 MB DMAs, 4 KB lines); bd fully resident.
  Sim profile: DMA 96% busy (roofline), PE/DVE/ACT balanced at ~56%.
"""
import os
import sys
from contextlib import ExitStack

for _p in ("/opt/trn_rl_repo",):
    if os.path.isdir(_p) and _p not in sys.path:
        sys.path.insert(0, _p)

import numpy as np

N_FULL = 4096
B_FULL = 512
D = 64
NCORES = 8
NS = N_FULL // NCORES        # rows per core

_built = {}


def _body(ctx, tc, y_d, x_d, bd_d, *, NS, B, SG, SC, repeat=1,
          split_copies=False, compact_bd=False, xg_bufs=2, yg_bufs=2):
    import concourse.mybir as mybir
    from concourse.masks import make_identity

    nc = tc.nc
    f32 = mybir.dt.float32
    bf16 = mybir.dt.bfloat16
    S = B // 2               # strips (2 blocks each)
    NT = NS // 128           # n-tiles
    G = S // SG              # strip groups (x/y DMA granularity)
    CG = SG * 128            # columns per group
    BC = S // SC             # bd chunks

    const_pool = ctx.enter_context(tc.tile_pool(name="const", bufs=1))
    xg_pool = ctx.enter_context(tc.tile_pool(name="xg", bufs=xg_bufs))
    yg_pool = ctx.enter_context(tc.tile_pool(name="yg", bufs=yg_bufs))
    bd_pool = ctx.enter_context(tc.tile_pool(name="bd", bufs=2))
    xt_pool = ctx.enter_context(tc.tile_pool(name="xt", bufs=3))
    psx_pool = ctx.enter_context(tc.tile_pool(name="psx", bufs=3, space="PSUM"))
    psy_pool = ctx.enter_context(tc.tile_pool(name="psy", bufs=3, space="PSUM"))
    if compact_bd:
        bdx_pool = ctx.enter_context(tc.tile_pool(name="bdx", bufs=3))

    ident_f = const_pool.tile([128, 128], f32)
    make_identity(nc, ident_f)
    ident = const_pool.tile([128, 128], bf16)
    nc.gpsimd.tensor_copy(ident[:], ident_f[:])

    if compact_bd:
        # bdm[64j+i, u, o] = 1 if u == j else 0 (block-diag expander mask)
        bdm = const_pool.tile([128, 2, D], bf16)
        nc.gpsimd.memset(bdm[:], 0.0)
        for j in range(2):
            nc.gpsimd.memset(bdm[64 * j:64 * j + 64, j, :], 1.0)

    x_view = x_d.rearrange("(t p) c -> p t c", p=128)     # [128, NT, C]
    y_view = y_d.rearrange("(t p) c -> p t c", p=128)

    xg_tiles = {}
    bd_tiles = {}
    yg_tiles = {}

    def fetch_group(g):
        xg = xg_pool.tile([128, NT, CG], bf16, tag="xg")
        nc.sync.dma_start(xg[:], x_view[:, :, g * CG:(g + 1) * CG])
        xg_tiles[g] = xg

    def fetch_bd(c):
        if compact_bd:
            bd = bd_pool.tile([128, SC, D], bf16, tag="bd")
        else:
            bd = bd_pool.tile([128, SC, 128], bf16, tag="bd")
        nc.sync.dma_start(bd[:], bd_d[:, c * SC:(c + 1) * SC, :])
        bd_tiles[c] = bd

    def stage_a(s):
        g, sl = divmod(s, SG)
        if sl == 0 and g + 1 < G:
            fetch_group(g + 1)
        if s % SC == SC // 2 and s // SC + 1 < BC:
            fetch_bd(s // SC + 1)
        xg = xg_tiles[g]
        psx = psx_pool.tile([128, NS], bf16, tag="psx")
        for t in range(NT):
            nc.tensor.matmul(psx[:, t * 128:(t + 1) * 128],
                             lhsT=xg[:, t, sl * 128:(sl + 1) * 128],
                             rhs=ident[:], is_transpose=True,
                             start=(t == 0), stop=(t == NT - 1))
        xt = xt_pool.tile([128, NS], bf16, tag="xt")
        h = NS // 2
        if split_copies:
            if s % 2 == 0:
                nc.vector.tensor_copy(xt[:, :h], psx[:, :h])
                nc.scalar.copy(xt[:, h:], psx[:, h:])
            else:
                nc.scalar.copy(xt[:, :h], psx[:, :h])
                nc.vector.tensor_copy(xt[:, h:], psx[:, h:])
        elif s % 2 == 0:
            nc.vector.tensor_copy(xt[:], psx[:])
        else:
            nc.scalar.copy(xt[:], psx[:])
        bdx = None
        if compact_bd:
            lt = bd_tiles[s // SC]
            slc = s % SC
            bdx = bdx_pool.tile([128, 2, D], bf16, tag="bdx")
            nc.gpsimd.tensor_tensor(
                bdx[:], lt[:, slc, None, :].to_broadcast((128, 2, D)),
                bdm[:], op=mybir.AluOpType.mult)
        return xt, bdx

    def stage_b(s, xt, bdx):
        g, sl = divmod(s, SG)
        if sl == 0:
            yg = yg_pool.tile([128, NT, CG], bf16, tag="yg")
            yg_tiles[g] = yg
        yg = yg_tiles[g]
        if compact_bd:
            rhs = bdx.rearrange("p u c -> p (u c)")
        else:
            rhs = bd_tiles[s // SC][:, s % SC, :]
        psy = psy_pool.tile([128, NT, 128], f32, tag="psy")
        for t in range(NT):
            nc.tensor.matmul(psy[:, t, :],
                             lhsT=xt[:, t * 128:(t + 1) * 128],
                             rhs=rhs)
        dst = yg[:, :, sl * 128:(sl + 1) * 128]
        hh = NT // 2
        if split_copies:
            if s % 2 == 0:
                nc.scalar.copy(dst[:, :hh, :], psy[:, :hh, :])
                nc.vector.tensor_copy(dst[:, hh:, :], psy[:, hh:, :])
            else:
                nc.vector.tensor_copy(dst[:, :hh, :], psy[:, :hh, :])
                nc.scalar.copy(dst[:, hh:, :], psy[:, hh:, :])
        elif s % 2 == 0:
            nc.scalar.copy(dst, psy[:])
        else:
            nc.vector.tensor_copy(dst, psy[:])
        if sl == SG - 1:
            nc.sync.dma_start(y_view[:, :, g * CG:(g + 1) * CG], yg[:])

    for _rep in range(repeat):
        xg_tiles.clear()
        bd_tiles.clear()
        yg_tiles.clear()
        fetch_bd(0)
        fetch_group(0)
        prev = None
        for s in range(S):
            cur = stage_a(s)
            if prev is not None:
                stage_b(s - 1, *prev)
            prev = cur
        stage_b(S - 1, *prev)


def _body_cs(ctx, tc, y_d, x_d, bd_d, *, NT_ALL, B, SG, repeat=1,
             xg_bufs=3, yg_bufs=3, NSB=512):
    """Column-sharded body: this core owns B blocks (all N rows), bd resident.

    x_d/y_d are [NT_ALL*128, B*D]; rows processed in bands of NSB."""
    import concourse.mybir as mybir
    from concourse.masks import make_identity

    nc = tc.nc
    f32 = mybir.dt.float32
    bf16 = mybir.dt.bfloat16
    S = B // 2               # strips (2 blocks each)
    NT = NSB // 128          # n-tiles per band
    NB = NT_ALL // NT        # bands
    G = S // SG              # strip groups per band
    CG = SG * 128            # columns per group

    const_pool = ctx.enter_context(tc.tile_pool(name="const", bufs=1))
    xg_pool = ctx.enter_context(tc.tile_pool(name="xg", bufs=xg_bufs))
    yg_pool = ctx.enter_context(tc.tile_pool(name="yg", bufs=yg_bufs))
    bd_pool = ctx.enter_context(tc.tile_pool(name="bd", bufs=1))
    xt_pool = ctx.enter_context(tc.tile_pool(name="xt", bufs=3))
    psx_pool = ctx.enter_context(tc.tile_pool(name="psx", bufs=3, space="PSUM"))
    psy_pool = ctx.enter_context(tc.tile_pool(name="psy", bufs=3, space="PSUM"))

    ident_f = const_pool.tile([128, 128], f32)
    make_identity(nc, ident_f)
    ident = const_pool.tile([128, 128], bf16)
    nc.gpsimd.tensor_copy(ident[:], ident_f[:])

    x_view = x_d.rearrange("(b t p) c -> p b t c", p=128, t=NT)
    y_view = y_d.rearrange("(b t p) c -> p b t c", p=128, t=NT)

    xg_tiles = {}
    yg_tiles = {}
    bd_tile = [None]

    def fetch_group(b, g):
        xg = xg_pool.tile([128, NT, CG], bf16, tag="xg")
        nc.sync.dma_start(xg[:], x_view[:, b, :, g * CG:(g + 1) * CG])
        xg_tiles[(b, g)] = xg

    def stage_a(b, s):
        g, sl = divmod(s, SG)
        if sl == 0:
            if g + 1 < G:
                fetch_group(b, g + 1)
            elif b + 1 < NB:
                fetch_group(b + 1, 0)
        xg = xg_tiles[(b, g)]
        psx = psx_pool.tile([128, NSB], bf16, tag="psx")
        for t in range(NT):
            nc.tensor.matmul(psx[:, t * 128:(t + 1) * 128],
                             lhsT=xg[:, t, sl * 128:(sl + 1) * 128],
                             rhs=ident[:], is_transpose=True,
                             start=(t == 0), stop=(t == NT - 1))
        xt = xt_pool.tile([128, NSB], bf16, tag="xt")
        if s % 2 == 0:
            nc.vector.tensor_copy(xt[:], psx[:])
        else:
            nc.scalar.copy(xt[:], psx[:])
        return xt

    def stage_b(b, s, xt):
        g, sl = divmod(s, SG)
        if sl == 0:
            yg = yg_pool.tile([128, NT, CG], bf16, tag="yg")
            yg_tiles[(b, g)] = yg
        yg = yg_tiles[(b, g)]
        psy = psy_pool.tile([128, NT, 128], f32, tag="psy")
        for t in range(NT):
            nc.tensor.matmul(psy[:, t, :],
                             lhsT=xt[:, t * 128:(t + 1) * 128],
                             rhs=bd_tile[0][:, s, :])
        dst = yg[:, :, sl * 128:(sl + 1) * 128]
        if s % 2 == 0:
            nc.scalar.copy(dst, psy[:])
        else:
            nc.vector.tensor_copy(dst, psy[:])
        if sl == SG - 1:
            nc.sync.dma_start(y_view[:, b, :, g * CG:(g + 1) * CG], yg[:])

    for _rep in range(repeat):
        xg_tiles.clear()
        yg_tiles.clear()
        fetch_group(0, 0)
        bd = bd_pool.tile([128, S, 128], bf16, tag="bd")
        nc.sync.dma_start(bd[:], bd_d[:])
        bd_tile[0] = bd
        prev = None
        for b in range(NB):
            for s in range(S):
                cur = (b, s, stage_a(b, s))
                if prev is not None:
                    stage_b(prev[0], prev[1], prev[2])
                prev = cur
        stage_b(prev[0], prev[1], prev[2])


def build(NS=NS, B=B_FULL, SG=8, SC=32, repeat=1, split_copies=False,
          compact_bd=True, xg_bufs=3, yg_bufs=3):
    key = (NS, B, SG, SC, repeat, split_copies, compact_bd, xg_bufs, yg_bufs)
    if key in _built:
        return _built[key]
    import concourse.tile as tile
    import concourse.mybir as mybir
    from concourse import bacc

    bf16 = mybir.dt.bfloat16
    C = B * D
    S = B // 2
    nc = bacc.Bacc("TRN2", target_bir_lowering=False, debug=False)
    x_d = nc.dram_tensor("x", [NS, C], bf16, kind="ExternalInput").ap()
    bd_shape = [128, S, D] if compact_bd else [128, S, 128]
    bd_d = nc.dram_tensor("bd", bd_shape, bf16, kind="ExternalInput").ap()
    y_d = nc.dram_tensor("y", [NS, C], bf16, kind="ExternalOutput").ap()
    with tile.TileContext(nc) as tc, ExitStack() as ctx:
        _body(ctx, tc, y_d, x_d, bd_d, NS=NS, B=B, SG=SG, SC=SC, repeat=repeat,
              split_copies=split_copies, compact_bd=compact_bd,
              xg_bufs=xg_bufs, yg_bufs=yg_bufs)
    nc.compile()
    _built[key] = nc
    return nc


def _body_cs2(ctx, tc, y_d, x_d, bd_d, *, NT_ALL, B, SG, repeat=1,
              xg_bufs=3, yg_bufs=3, NSB=512, dve_frac=7, ragged=1, ysplit=0):
    """Paired-strip column-sharded body: strips processed two at a time so
    each PSUM->SBUF drain is one instruction (halves DVE/ACT fixed costs).

    xt2 copies always on DVE (bf16 PSUM reads are 2x there); psy2 copies on
    ACT except every dve_frac-th pair, which goes to DVE for balance."""
    import concourse.mybir as mybir
    from concourse.masks import make_identity

    nc = tc.nc
    f32 = mybir.dt.float32
    bf16 = mybir.dt.bfloat16
    S = B // 2               # strips (2 blocks each)
    NT = NSB // 128          # n-tiles per band
    NB = NT_ALL // NT        # bands
    G = S // SG              # strip groups per band
    CG = SG * 128            # columns per group
    assert S % 2 == 0 and SG % 2 == 0

    const_pool = ctx.enter_context(tc.tile_pool(name="const", bufs=1))
    xg_pool = ctx.enter_context(tc.tile_pool(name="xg", bufs=xg_bufs))
    yg_pool = ctx.enter_context(tc.tile_pool(name="yg", bufs=yg_bufs))
    bd_pool = ctx.enter_context(tc.tile_pool(name="bd", bufs=1))
    xt_pool = ctx.enter_context(tc.tile_pool(name="xt", bufs=3))
    psx_pool = ctx.enter_context(tc.tile_pool(name="psx", bufs=3, space="PSUM"))
    psy_pool = ctx.enter_context(tc.tile_pool(name="psy", bufs=2, space="PSUM"))

    ident_f = const_pool.tile([128, 128], f32)
    make_identity(nc, ident_f)
    ident = const_pool.tile([128, 128], bf16)
    nc.gpsimd.tensor_copy(ident[:], ident_f[:])

    x_view = x_d.rearrange("(b t p) c -> p b t c", p=128, t=NT)
    y_view = y_d.rearrange("(b t p) c -> p b t c", p=128, t=NT)

    # per-band group plans [(start_strip, width_strips)]; ragged halves the
    # first groups of band 0 and the last groups of the final band so the
    # pipeline-fill DMA (and final drain) is half-sized.
    def make_plan(b):
        plan = [(i * SG, SG) for i in range(G)]
        if ragged and SG >= 4:
            h = SG // 2
            if ragged >= 2 and SG >= 8:  # 2/3: both ends; 4: front; 5: tail
                q = SG // 4
                if b == 0 and ragged != 5:
                    plan = [(0, q), (q, q), (2 * q, h)] + plan[1:]
                if b == NB - 1 and ragged != 4:
                    plan = plan[:-1] + [(S - SG, h), (S - h, q), (S - q, q)]
            else:
                if b == 0:
                    plan = [(0, h), (h, h)] + plan[1:]
                if b == NB - 1:
                    plan = plan[:-1] + [(S - SG, h), (S - h, h)]
        return plan

    plans = [make_plan(b) for b in range(NB)]
    xg_burst = xg_bufs - 1
    smap = [{s: gi for gi, (c0, w) in enumerate(p)
             for s in range(c0, c0 + w)} for p in plans]

    xg_tiles = {}
    yg_tiles = {}
    bd_tile = [None]

    def fetch_group(b, gi):
        c0, w = plans[b][gi]
        xg = xg_pool.tile([128, NT, w * 128], bf16, tag="xg")
        nc.sync.dma_start(xg[:], x_view[:, b, :, c0 * 128:(c0 + w) * 128])
        xg_tiles[(b, gi)] = xg

    def stage_a(b, sp):
        """Transpose strips 2sp, 2sp+1 into one psx2; one copy to xt2."""
        psx = psx_pool.tile([128, 2, NSB], bf16, tag="psx")
        for j in range(2):
            s = 2 * sp + j
            gi = smap[b][s]
            c0, w = plans[b][gi]
            sl = s - c0
            if sl == 0:
                if ragged >= 3 and b == 0 and gi == 0:
                    for gq in range(1, min(xg_burst, len(plans[0]))):
                        fetch_group(0, gq)
                elif gi + 1 < len(plans[b]):
                    if (b, gi + 1) not in xg_tiles:
                        fetch_group(b, gi + 1)
                elif b + 1 < NB:
                    fetch_group(b + 1, 0)
            xg = xg_tiles[(b, gi)]
            for t in range(NT):
                nc.tensor.matmul(psx[:, j, t * 128:(t + 1) * 128],
                                 lhsT=xg[:, t, sl * 128:(sl + 1) * 128],
                                 rhs=ident[:], is_transpose=True,
                                 start=(j == 0 and t == 0),
                                 stop=(j == 1 and t == NT - 1))
        xt = xt_pool.tile([128, 2, NSB], bf16, tag="xt")
        nc.vector.tensor_copy(xt[:], psx[:])
        return xt

    def stage_b(b, sp, xt):
        psy = psy_pool.tile([128, 2, NT, 128], f32, tag="psy")
        for j in range(2):
            s = 2 * sp + j
            gi = smap[b][s]
            c0, w = plans[b][gi]
            if s == c0:
                yg = yg_pool.tile([128, NT, w * 128], bf16, tag="yg")
                yg_tiles[(b, gi)] = yg
            for t in range(NT):
                nc.tensor.matmul(psy[:, j, t, :],
                                 lhsT=xt[:, j, t * 128:(t + 1) * 128],
                                 rhs=bd_tile[0][:, s, :])
        g0 = smap[b][2 * sp]
        c0, w = plans[b][g0]
        sl0 = 2 * sp - c0
        yg = yg_tiles[(b, g0)]
        dst = yg[:, :, sl0 * 128:(sl0 + 2) * 128].rearrange(
            "p t (j c) -> p j t c", j=2)
        if sp % dve_frac == 0:
            nc.vector.tensor_copy(dst, psy[:])
        else:
            nc.scalar.copy(dst, psy[:])
        if ysplit and w >= 8:
            hw_ = w // 2
            if sl0 + 2 == hw_:
                nc.sync.dma_start(y_view[:, b, :, c0 * 128:(c0 + hw_) * 128],
                                  yg[:, :, :hw_ * 128])
            elif sl0 + 2 == w:
                nc.sync.dma_start(
                    y_view[:, b, :, (c0 + hw_) * 128:(c0 + w) * 128],
                    yg[:, :, hw_ * 128:])
        elif sl0 + 2 == w:
            nc.sync.dma_start(y_view[:, b, :, c0 * 128:(c0 + w) * 128], yg[:])

    for _rep in range(repeat):
        xg_tiles.clear()
        yg_tiles.clear()
        fetch_group(0, 0)
        bd = bd_pool.tile([128, S, 128], bf16, tag="bd")
        nc.sync.dma_start(bd[:], bd_d[:])
        bd_tile[0] = bd
        prev = None
        for b in range(NB):
            for sp in range(S // 2):
                cur = (b, sp, stage_a(b, sp))
                if prev is not None:
                    stage_b(prev[0], prev[1], prev[2])
                prev = cur
        stage_b(prev[0], prev[1], prev[2])


def _body_diag(ctx, tc, y_d, x_d, bd_d, *, NT_ALL, B, SG, repeat=1,
               xg_bufs=3, yg_bufs=3, NSB=512, mode="dma", dve_frac=7):
    """Diagnostic bodies: mode='dma' issues only the DMA traffic of the real
    kernel; mode='nodma' runs the full compute/copy pipeline with all
    dma_start calls elided (tiles allocated, never filled/drained)."""
    import concourse.mybir as mybir
    from concourse.masks import make_identity

    nc = tc.nc
    f32 = mybir.dt.float32
    bf16 = mybir.dt.bfloat16
    S = B // 2
    NT = NSB // 128
    NB = NT_ALL // NT
    G = S // SG
    CG = SG * 128

    const_pool = ctx.enter_context(tc.tile_pool(name="const", bufs=1))
    xg_pool = ctx.enter_context(tc.tile_pool(name="xg", bufs=xg_bufs))
    yg_pool = ctx.enter_context(tc.tile_pool(name="yg", bufs=yg_bufs))
    bd_pool = ctx.enter_context(tc.tile_pool(name="bd", bufs=1))
    xt_pool = ctx.enter_context(tc.tile_pool(name="xt", bufs=3))
    psx_pool = ctx.enter_context(tc.tile_pool(name="psx", bufs=3, space="PSUM"))
    psy_pool = ctx.enter_context(tc.tile_pool(name="psy", bufs=2, space="PSUM"))

    ident_f = const_pool.tile([128, 128], f32)
    make_identity(nc, ident_f)
    ident = const_pool.tile([128, 128], bf16)
    nc.gpsimd.tensor_copy(ident[:], ident_f[:])

    x_view = x_d.rearrange("(b t p) c -> p b t c", p=128, t=NT)
    y_view = y_d.rearrange("(b t p) c -> p b t c", p=128, t=NT)

    do_dma = mode == "dma"
    xg_tiles = {}
    yg_tiles = {}
    bd_tile = [None]
    if mode == "dma":
        yg0 = const_pool.tile([128, NT, CG], bf16)
        nc.gpsimd.memset(yg0[:], 0.0)
    xg0 = bd0 = None
    if mode == "nodma":
        xg0 = const_pool.tile([128, NT, CG], bf16)
        nc.gpsimd.memset(xg0[:], 0.0)
        bd0 = const_pool.tile([128, S, 128], bf16)
        nc.gpsimd.memset(bd0[:], 0.0)

    def fetch_group(b, g):
        if not do_dma:
            xg_tiles[(b, g)] = xg0
            return
        xg = xg_pool.tile([128, NT, CG], bf16, tag="xg")
        nc.sync.dma_start(xg[:], x_view[:, b, :, g * CG:(g + 1) * CG])
        xg_tiles[(b, g)] = xg

    def stage_a(b, sp):
        psx = psx_pool.tile([128, 2, NSB], bf16, tag="psx")
        for j in range(2):
            s = 2 * sp + j
            g, sl = divmod(s, SG)
            if sl == 0:
                if g + 1 < G:
                    fetch_group(b, g + 1)
                elif b + 1 < NB:
                    fetch_group(b + 1, 0)
            xg = xg_tiles[(b, g)]
            for t in range(NT):
                nc.tensor.matmul(psx[:, j, t * 128:(t + 1) * 128],
                                 lhsT=xg[:, t, sl * 128:(sl + 1) * 128],
                                 rhs=ident[:], is_transpose=True,
                                 start=(j == 0 and t == 0),
                                 stop=(j == 1 and t == NT - 1))
        xt = xt_pool.tile([128, 2, NSB], bf16, tag="xt")
        nc.vector.tensor_copy(xt[:], psx[:])
        return xt

    def stage_b(b, sp, xt):
        psy = psy_pool.tile([128, 2, NT, 128], f32, tag="psy")
        for j in range(2):
            s = 2 * sp + j
            g, sl = divmod(s, SG)
            if sl == 0:
                yg = yg_pool.tile([128, NT, CG], bf16, tag="yg")
                yg_tiles[(b, g)] = yg
            for t in range(NT):
                nc.tensor.matmul(psy[:, j, t, :],
                                 lhsT=xt[:, j, t * 128:(t + 1) * 128],
                                 rhs=bd_tile[0][:, s, :])
        g0, sl0 = divmod(2 * sp, SG)
        yg = yg_tiles[(b, g0)]
        dst = yg[:, :, sl0 * 128:(sl0 + 2) * 128].rearrange(
            "p t (j c) -> p j t c", j=2)
        if sp % dve_frac == 0:
            nc.vector.tensor_copy(dst, psy[:])
        else:
            nc.scalar.copy(dst, psy[:])
        if do_dma and sl0 + 2 == SG:
            nc.sync.dma_start(y_view[:, b, :, g0 * CG:(g0 + 1) * CG], yg[:])

    for _rep in range(repeat):
        xg_tiles.clear()
        yg_tiles.clear()
        fetch_group(0, 0)
        if do_dma:
            bd = bd_pool.tile([128, S, 128], bf16, tag="bd")
            nc.sync.dma_start(bd[:], bd_d[:])
            bd_tile[0] = bd
        else:
            bd_tile[0] = bd0
        if mode == "dma":
            for b in range(NB):
                for g in range(G):
                    if (b, g) != (0, 0):
                        fetch_group(b, g)
                    nc.sync.dma_start(y_view[:, b, :, g * CG:(g + 1) * CG],
                                      yg0[:])
            continue
        prev = None
        for b in range(NB):
            for sp in range(S // 2):
                cur = (b, sp, stage_a(b, sp))
                if prev is not None:
                    stage_b(prev[0], prev[1], prev[2])
                prev = cur
        stage_b(prev[0], prev[1], prev[2])


def build_cs(N=N_FULL, BL=B_FULL // NCORES, SG=16, repeat=1, xg_bufs=4,
             yg_bufs=4, NSB=512, paired=1, dve_frac=7, diag=None,
             ragged=1, ysplit=0):
    key = ("cs", N, BL, SG, repeat, xg_bufs, yg_bufs, NSB, paired, dve_frac,
           diag, ragged, ysplit)
    if key in _built:
        return _built[key]
    import concourse.tile as tile
    import concourse.mybir as mybir
    from concourse import bacc

    bf16 = mybir.dt.bfloat16
    C = BL * D
    S = BL // 2
    nc = bacc.Bacc("TRN2", target_bir_lowering=False, debug=False)
    x_d = nc.dram_tensor("x", [N, C], bf16, kind="ExternalInput").ap()
    bd_d = nc.dram_tensor("bd", [128, S, 128], bf16, kind="ExternalInput").ap()
    y_d = nc.dram_tensor("y", [N, C], bf16, kind="ExternalOutput").ap()
    body = _body_cs2 if paired else _body_cs
    kw = dict(dve_frac=dve_frac) if paired else {}
    if paired and not diag:
        kw["ragged"] = ragged
        kw["ysplit"] = ysplit
    if diag:
        body = _body_diag
        kw = dict(mode=diag, dve_frac=dve_frac)
    with tile.TileContext(nc) as tc, ExitStack() as ctx:
        body(ctx, tc, y_d, x_d, bd_d, NT_ALL=N // 128, B=BL, SG=SG,
             repeat=repeat, xg_bufs=xg_bufs, yg_bufs=yg_bufs, NSB=NSB, **kw)
    nc.compile()
    _built[key] = nc
    return nc


def make_core_inputs(xb, bd, col_shard=False):
    """Per-core input dicts from full host-prepped arrays."""
    if not col_shard:
        ns = xb.shape[0] // NCORES
        return [{"x": xb[k * ns:(k + 1) * ns], "bd": bd} for k in range(NCORES)]
    cl = xb.shape[1] // NCORES
    sl = bd.shape[1] // NCORES
    return [{"x": np.ascontiguousarray(xb[:, k * cl:(k + 1) * cl]),
             "bd": np.ascontiguousarray(bd[:, k * sl:(k + 1) * sl, :])}
            for k in range(NCORES)]


def unshard_y(ys, col_shard=False):
    return np.concatenate(ys, axis=1 if col_shard else 0)


def host_prep(x, weights, B=B_FULL, compact_bd=True):
    """x [N, B*D] f32, weights [B, D, D] f32 ->
    (x bf16 [N, B*D], bd bf16 [128, B//2, 128] or compact [128, B//2, 64])."""
    import ml_dtypes

    bf = ml_dtypes.bfloat16
    S = B // 2
    w = np.ascontiguousarray(weights, dtype=np.float32)
    L = np.tril(w, -1)
    idx = np.arange(D)
    L[:, idx, idx] = np.exp(w[:, idx, idx])
    # LTr[j, i, s, o] = L[2s+j, o, i]
    LTr = L.reshape(S, 2, D, D).transpose(1, 3, 0, 2)
    if compact_bd:
        # lt[64j+i, s, o] = L_{2s+j}[o, i]
        bd = np.ascontiguousarray(
            LTr.reshape(2 * D, S, D).astype(bf))
    else:
        bd = np.zeros((2, D, S, 2, D), dtype=bf)
        bd[0, :, :, 0, :] = LTr[0].astype(bf)
        bd[1, :, :, 1, :] = LTr[1].astype(bf)
        bd = np.ascontiguousarray(bd.reshape(2 * D, S, 2 * D))
    xb = np.ascontiguousarray(np.asarray(x, dtype=np.float32)).astype(bf)
    return xb, bd


def _pin_compile_cache(extra=""):
    import hashlib
    with open(os.path.abspath(__file__), "rb") as f:
        h = hashlib.sha256(f.read() + extra.encode()).hexdigest()[:16]
    os.environ["NEURON_COMPILE_CACHE_URL"] = f"/tmp/neuron_cache_{h}"


def run(x, weights, trace=False, col_shard=True, **opts):
    from concourse import bass_utils

    _pin_compile_cache()

    x = np.asarray(x)
    weights = np.asarray(weights)
    assert x.shape == (N_FULL, B_FULL * D), x.shape
    assert weights.shape == (B_FULL, D, D), weights.shape
    xb, bd = host_prep(x, weights,
                       compact_bd=(not col_shard) and opts.get("compact_bd", True))

    nc = build_cs(**opts) if col_shard else build(**opts)
    in_maps = make_core_inputs(xb, bd, col_shard=col_shard)
    res = bass_utils.run_bass_kernel_spmd(
        nc, in_maps, core_ids=list(range(NCORES)), trace=trace)
    y = unshard_y([np.asarray(res.results[k]["y"]) for k in range(NCORES)],
                  col_shard=col_shard)
    return y.astype(np.float32), res


def kernel(x, weights):
    y, _ = run(x, weights)
    return y
